# revision 30
# baseline (speedup 1.0000x reference)
"""Trainium2 Bass kernel for nn_MatrixLSTMCell (mLSTM, parallel stabilized).

Sharding: 8 cores = (batch b in 0..3) x (head-group g in 0..1), 6 heads/core.

Math (equivalent chunked linear-attention form of the reference):
  L[s] = cumsum(log_sigmoid(fg))[s],  m[j] = ig[j] - L[j],  M = cummax(m),
  cH = M[S-1],  em[j] = 0.125 * exp(m[j] - cH)
  ph[i] = sum_{j<=i} (q_i . k_j) * em[j] * [v_j | 1]      (device, O(S^2))
  h[i]  = ph_v[i] / (max(|ph_rs[i]|, exp(-L-cH)) + eps*exp(M-cH))
then per-head groupnorm over dh (host epilogue; scan/gates also host: O(S)).

Device: per 128-row chunk r the causal sum splits into an intra-chunk
masked attention (6 heads' [128,128] qk^T packed in PSUM, one tril
mask-multiply on DVE) plus a running state W = sum_j k_j em_j [v_j|1]^T
applied as q @ W.  em folds into va = [v|1]*em once per chunk so the
state update consumes raw k (no per-head elementwise work).  The loop is
software-pipelined one chunk ahead so Tensor/Vector/Scalar never stall
on same-chunk work.
"""

import numpy as np
import ml_dtypes

import concourse.bass as bass
import concourse.bacc as bacc
import concourse.mybir as mybir
import concourse.tile as tile
from concourse.bass_utils import run_bass_kernel_spmd

F32 = mybir.dt.float32
BF16 = mybir.dt.bfloat16
AF = mybir.ActivationFunctionType
OP = mybir.AluOpType

B, S, DIM = 4, 1024, 768
NH, DH = 12, 64
HPC = 6                # heads per core
DA = DH + 1            # v augmented with a ones column
NCH = S // 128         # 8 chunks


def build_nc():
    nc = bacc.Bacc(None, target_bir_lowering=False)
    qs = nc.dram_tensor("qs", [64, NCH * 2 * HPC * 128], BF16,
                        kind="ExternalInput")[:]
    kn = nc.dram_tensor("kn", [128, NCH * HPC * DH], BF16,
                        kind="ExternalInput")[:]
    va = nc.dram_tensor("va", [128, NCH * HPC * DA], BF16,
                        kind="ExternalInput")[:]
    mk = nc.dram_tensor("mk", [128, 128], BF16, kind="ExternalInput")[:]
    out = nc.dram_tensor("out", [128, NCH * HPC * DA], BF16,
                         kind="ExternalOutput")[:]
    with tile.TileContext(nc) as tc:
        with tc.tile_pool(name="persist", bufs=1) as persist:
            _body(nc, tc, persist, qs, kn, va, mk, out)
    nc.finalize()
    return nc


def _body(nc, tc, persist, qs, kn, va, mk, out):
    # persistent SBUF inputs
    qs_sb = persist.tile([64, NCH, 2 * HPC, 128], BF16)   # slot 2h=q_h, 2h+1=k_h
    kn_sb = persist.tile([128, NCH, HPC * DH], BF16)      # position-major k
    va_sb = persist.tile([128, NCH, HPC * DA], BF16)      # [v|1]*em per head
    mk_sb = persist.tile([128, 128], BF16)                # tril(1) mask

    scratch = persist.tile([128, 512], BF16)              # PE warm-up feed

    qs_c = qs.rearrange("p (c x) -> p c x", c=NCH)
    kn_c = kn.rearrange("p (c x) -> p c x", c=NCH)
    va_c = va.rearrange("p (c x) -> p c x", c=NCH)
    out_c = out.rearrange("p (c x) -> p c x", c=NCH)

    # input DMAs spread over three DGE queues (sync, scalar-free gpsimd
    # software DGE) so descriptor issue (~0.6us each) pipelines and the
    # scalar queue stays free for the per-chunk W drains
    qs_r = qs_c.rearrange("p c (h s) -> p c h s", h=2 * HPC)
    for c in range(NCH):
        nc.sync.dma_start(out=qs_sb[:, c:c + 1], in_=qs_r[:, c:c + 1])
    nc.gpsimd.dma_start(out=mk_sb[:], in_=mk)
    nc.gpsimd.dma_start(out=va_sb[:, 0:1], in_=va_c[:, 0:1])
    nc.gpsimd.dma_start(out=kn_sb[:, 0:1], in_=kn_c[:, 0:1])
    nc.gpsimd.dma_start(out=va_sb[:, 1:2], in_=va_c[:, 1:2])
    nc.gpsimd.dma_start(out=kn_sb[:, 1:2], in_=kn_c[:, 1:2])
    nc.gpsimd.dma_start(out=va_sb[:, 2:4], in_=va_c[:, 2:4])
    nc.gpsimd.dma_start(out=kn_sb[:, 2:4], in_=kn_c[:, 2:4])
    nc.gpsimd.dma_start(out=va_sb[:, 4:8], in_=va_c[:, 4:8])
    nc.gpsimd.dma_start(out=kn_sb[:, 4:8], in_=kn_c[:, 4:8])

    nc.vector.memset(scratch[:], 0.0)

    with (
        tc.tile_pool(name="psQK", bufs=2, space="PSUM") as psQK,
        tc.tile_pool(name="psH", bufs=2, space="PSUM") as psH,
        tc.tile_pool(name="psW", bufs=1, space="PSUM") as psW,
        tc.tile_pool(name="psWarm", bufs=1, space="PSUM") as psWarm,
        tc.tile_pool(name="work", bufs=2) as work,
    ):
        # all PSUM tiles are exact bank multiples so tiles never share a
        # bank (a matmul start=True clears the whole bank's has_written)
        psum_W = psW.tile([128, 512], F32)
        wview = psum_W[0:64, 0:HPC * DA].rearrange("p (h d) -> p h d", h=HPC)

        # HAM warm-up: the PE clock sits at 1.2 GHz until ~3.4us of
        # sustained matmul activity.  Spend the DMA-bound prologue on
        # dummy matmuls so real work starts (and stays) at 2.4 GHz.
        warm = psWarm.tile([128, 512], F32)
        for _ in range(10):
            nc.tensor.matmul(warm[:], lhsT=scratch[:, 0:128], rhs=scratch[:],
                             start=True, stop=True, skip_group_check=True)

        def emit_pqk(r):
            pq = psQK.tile([128, 1024], F32, name="pqk")
            for h in range(HPC):
                nc.tensor.matmul(pq[:, h * 128:(h + 1) * 128],
                                 lhsT=qs_sb[:, r, 2 * h + 1, :],
                                 rhs=qs_sb[:, r, 2 * h, :],
                                 start=True, stop=True, skip_group_check=True)
            return pq

        def emit_cp(pq):
            t = work.tile([128, HPC, 128], BF16, name="cp")
            nc.vector.tensor_tensor(
                out=t[:],
                in0=pq[:, 0:HPC * 128].rearrange("p (h s) -> p h s", h=HPC),
                in1=mk_sb[:].unsqueeze(1).broadcast_to([128, HPC, 128]),
                op=OP.mult)
            return t

        cp_cur = emit_cp(emit_pqk(0))
        wsb_prev = None

        for r in range(NCH):
            # bridge PE idle gaps so the HAM clock gate never sees an idle
            # window and re-throttles to 1.2GHz
            for _ in range(2):
                nc.tensor.matmul(warm[:, 0:256], lhsT=scratch[:, 0:128],
                                 rhs=scratch[:, 0:256], start=True, stop=True,
                                 skip_group_check=True)
            if r + 1 < NCH:
                pq_n = emit_pqk(r + 1)       # tensor works ahead one chunk
                cp_nxt = emit_cp(pq_n)
            ph = psH.tile([128, 512], F32, name="ph")
            phv = ph[:, 0:HPC * DA].rearrange("p (h d) -> p h d", h=HPC)
            va_r = va_sb[:, r].rearrange("p (h d) -> p h d", h=HPC)
            if r > 0:
                # inter-chunk: ph = q @ W_{<r}; h==0 claims the bank
                for h in range(HPC):
                    nc.tensor.matmul(phv[:, h, :],
                                     lhsT=qs_sb[:, r, 2 * h, :],
                                     rhs=wsb_prev[:, h, :],
                                     start=(h == 0), stop=False,
                                     skip_group_check=True)
            for h in range(HPC):
                nc.tensor.matmul(phv[:, h, :], lhsT=cp_cur[:, h, :],
                                 rhs=va_r[:, h, :],
                                 start=(r == 0 and h == 0), stop=True,
                                 skip_group_check=True)
            if r + 1 < NCH:
                # (the last chunk's state update is dead work: W unread)
                for h in range(HPC):
                    nc.tensor.matmul(wview[:, h, :],
                                     lhsT=kn_sb[:, r, h * DH:(h + 1) * DH],
                                     rhs=va_r[:, h, :],
                                     start=(r == 0 and h == 0),
                                     stop=(r == NCH - 2), skip_group_check=True)
                # drain W on scalar+vector in parallel: next chunk's inter
                # group is gated on this, so latency matters
                wsb = work.tile([64, HPC, DA], BF16, name="wsb")
                nc.scalar.activation(out=wsb[:, 0:3], in_=wview[:, 0:3],
                                     func=AF.Copy)
                nc.vector.tensor_copy(out=wsb[:, 3:6], in_=wview[:, 3:6])
            phsb = work.tile([128, HPC * DA], BF16, name="phsb")
            if r + 1 < NCH:
                nc.vector.tensor_copy(out=phsb[:], in_=ph[:, 0:HPC * DA])
                nc.sync.dma_start(out=out_c[:, r], in_=phsb[:])
            else:
                # split the final drain + out DMA for a shorter tail
                nc.vector.tensor_copy(out=phsb[:, 0:3 * DA],
                                      in_=ph[:, 0:3 * DA])
                nc.sync.dma_start(out=out_c[:, r, 0:3 * DA],
                                  in_=phsb[:, 0:3 * DA])
                nc.vector.tensor_copy(out=phsb[:, 3 * DA:HPC * DA],
                                      in_=ph[:, 3 * DA:HPC * DA])
                nc.sync.dma_start(out=out_c[:, r, 3 * DA:HPC * DA],
                                  in_=phsb[:, 3 * DA:HPC * DA])
            if r + 1 < NCH:
                cp_cur, wsb_prev = cp_nxt, wsb


_CACHED_NC = None


def _get_nc():
    global _CACHED_NC
    if _CACHED_NC is None:
        _CACHED_NC = build_nc()
    return _CACHED_NC


def _host_gates(q, k, v, igate_w, igate_b, fgate_w, fgate_b):
    """O(S) gate/scan work on host: returns em (bf16-ready), e2/emp, eps/emp."""
    x = np.concatenate([q, k, v], axis=2).reshape(-1, 3 * DIM)   # f32 gemm
    ig = (x @ igate_w.T).reshape(B, S, NH).astype(np.float64) + igate_b
    fg = (x @ fgate_w.T).reshape(B, S, NH).astype(np.float64) + fgate_b
    ls = -np.logaddexp(0.0, -fg)                 # log sigmoid
    L = np.cumsum(ls, axis=1)
    m = ig - L
    Mx = np.maximum.accumulate(m, axis=1)
    cH = Mx[:, -1:, :]
    em = np.exp(m - cH) * 0.125                  # <= 0.125, no overflow
    e2e = np.exp(-L - cH)                        # e2/emp (exponent bounded)
    epse = 1e-6 * np.exp(Mx - cH)                # eps/emp <= 1e-6
    return em, e2e, epse


_MASK_HOST = np.ascontiguousarray(
    np.tril(np.ones((128, 128), np.float32)).T).astype(ml_dtypes.bfloat16)


def _prep_core(q, k, v, em, b, g):
    hs = slice(HPC * g, HPC * g + HPC)
    qh = q[b].reshape(S, NH, DH)[:, hs]          # [S, 6, 64]
    kh = k[b].reshape(S, NH, DH)[:, hs]
    vh = v[b].reshape(S, NH, DH)[:, hs]
    qk2 = np.stack([qh, kh], axis=2)             # [S, 6, 2, 64]
    qs_host = np.ascontiguousarray(
        qk2.reshape(NCH, 128, HPC, 2, DH).transpose(4, 0, 2, 3, 1)
    ).reshape(64, -1).astype(ml_dtypes.bfloat16)
    kn_host = np.ascontiguousarray(
        kh.reshape(NCH, 128, HPC * DH).transpose(1, 0, 2)
    ).reshape(128, -1).astype(ml_dtypes.bfloat16)
    va = np.ones((NCH, 128, HPC, DA), np.float32)
    va[..., :DH] = vh.reshape(NCH, 128, HPC, DH)
    va *= em[b][:, hs].reshape(NCH, 128, HPC, 1)   # fold 0.125*exp(m-cH)
    va_host = np.ascontiguousarray(
        va.transpose(1, 0, 2, 3)).reshape(128, -1).astype(ml_dtypes.bfloat16)
    return {"qs": qs_host, "kn": kn_host, "va": va_host, "mk": _MASK_HOST}


_LAST_RESULT = {}


def kernel(q, k, v, igate_w, igate_b, fgate_w, fgate_b, norm_w, norm_b,
           **run_kwargs):
    nc = _get_nc()
    em, e2e, epse = _host_gates(q, k, v, igate_w, igate_b, fgate_w, fgate_b)
    in_maps = [_prep_core(q, k, v, em, core // 2, core % 2)
               for core in range(8)]

    res = run_bass_kernel_spmd(nc, in_maps, core_ids=list(range(8)),
                               **run_kwargs)
    _LAST_RESULT["res"] = res

    out = np.zeros((B, S, NH, DH), np.float32)
    for core in range(8):
        b, g = core // 2, core % 2
        hs = slice(HPC * g, HPC * g + HPC)
        o = np.asarray(res.results[core]["out"], dtype=np.float64)
        o = o.reshape(128, NCH, HPC, DA).transpose(1, 0, 2, 3).reshape(
            S, HPC, DA)
        ph_v, ph_rs = o[:, :, :DH], o[:, :, DH]
        sc = 1.0 / (np.maximum(np.abs(ph_rs), e2e[b][:, hs]) + epse[b][:, hs])
        h = ph_v * sc[..., None]
        mean = h.mean(-1, keepdims=True)
        var = ((h - mean) ** 2).mean(-1, keepdims=True)
        out[b, :, hs] = (h - mean) / np.sqrt(var + 1e-5)

    out = out.reshape(B, S, DIM)
    if np.any(norm_w) or np.any(norm_b):
        out = out * (1.0 + norm_w)[None, None, :] + norm_b[None, None, :]
    return out


# revision 32
# speedup vs baseline: 1.1092x; 1.1092x over previous
"""Trainium2 Bass kernel for nn_MatrixLSTMCell (mLSTM, parallel stabilized).

Sharding: 8 cores = (batch b in 0..3) x (head-group g in 0..1), 6 heads/core.

Math (equivalent chunked linear-attention form of the reference):
  L[s] = cumsum(log_sigmoid(fg))[s],  m[j] = ig[j] - L[j],  M = cummax(m),
  cH = M[S-1],  em[j] = 0.125 * exp(m[j] - cH)
  ph[i] = sum_{j<=i} (q_i . k_j) * em[j] * [v_j | 1]      (device, O(S^2))
  h[i]  = ph_v[i] / (max(|ph_rs[i]|, exp(-L-cH)) + eps*exp(M-cH))
then per-head groupnorm over dh (host epilogue; scan/gates also host: O(S)).

Device: per 128-row chunk r the causal sum splits into an intra-chunk
masked attention (6 heads' [128,128] qk^T packed in PSUM, one tril
mask-multiply on DVE) plus a running state W = sum_j k_j em_j [v_j|1]^T
applied as q @ W.  em folds into va = [v|1]*em once per chunk so the
state update consumes raw k (no per-head elementwise work).  The loop is
software-pipelined one chunk ahead so Tensor/Vector/Scalar never stall
on same-chunk work.
"""

import numpy as np
import ml_dtypes

import concourse.bass as bass
import concourse.bacc as bacc
import concourse.mybir as mybir
import concourse.tile as tile
from concourse.bass_utils import run_bass_kernel_spmd

F32 = mybir.dt.float32
BF16 = mybir.dt.bfloat16
AF = mybir.ActivationFunctionType
OP = mybir.AluOpType

B, S, DIM = 4, 1024, 768
NH, DH = 12, 64
HPC = 6                # heads per core
DA = DH + 1            # v augmented with a ones column
NCH = S // 128         # 8 chunks


def build_nc():
    nc = bacc.Bacc(None, target_bir_lowering=False)
    qs = nc.dram_tensor("qs", [64, NCH * 2 * HPC * 128], BF16,
                        kind="ExternalInput")[:]
    kn = nc.dram_tensor("kn", [128, NCH * HPC * DH], BF16,
                        kind="ExternalInput")[:]
    va = nc.dram_tensor("va", [128, NCH * HPC * DA], BF16,
                        kind="ExternalInput")[:]
    mk = nc.dram_tensor("mk", [128, 128], BF16, kind="ExternalInput")[:]
    out = nc.dram_tensor("out", [128, NCH * HPC * DA], BF16,
                         kind="ExternalOutput")[:]
    with tile.TileContext(nc) as tc:
        with tc.tile_pool(name="persist", bufs=1) as persist:
            _body(nc, tc, persist, qs, kn, va, mk, out)
    nc.finalize()
    return nc


def _body(nc, tc, persist, qs, kn, va, mk, out):
    # persistent SBUF inputs
    qs_sb = persist.tile([64, NCH, 2 * HPC, 128], BF16)   # slot 2h=q_h, 2h+1=k_h
    kn_sb = persist.tile([128, NCH, HPC * DH], BF16)      # position-major k
    va_sb = persist.tile([128, NCH, HPC * DA], BF16)      # [v|1]*em per head
    mk_sb = persist.tile([128, 128], BF16)                # tril(1) mask

    scratch = persist.tile([128, 512], BF16)              # PE warm-up feed

    qs_c = qs.rearrange("p (c x) -> p c x", c=NCH)
    kn_c = kn.rearrange("p (c x) -> p c x", c=NCH)
    va_c = va.rearrange("p (c x) -> p c x", c=NCH)
    out_c = out.rearrange("p (c x) -> p c x", c=NCH)

    # input DMAs spread over three DGE queues (sync, scalar-free gpsimd
    # software DGE) so descriptor issue (~0.6us each) pipelines and the
    # scalar queue stays free for the per-chunk W drains
    qs_r = qs_c.rearrange("p c (h s) -> p c h s", h=2 * HPC)
    for c in range(NCH):
        nc.sync.dma_start(out=qs_sb[:, c:c + 1], in_=qs_r[:, c:c + 1])
    nc.gpsimd.dma_start(out=mk_sb[:], in_=mk)
    nc.gpsimd.dma_start(out=va_sb[:, 0:1], in_=va_c[:, 0:1])
    nc.gpsimd.dma_start(out=kn_sb[:, 0:1], in_=kn_c[:, 0:1])
    nc.gpsimd.dma_start(out=va_sb[:, 1:2], in_=va_c[:, 1:2])
    nc.gpsimd.dma_start(out=kn_sb[:, 1:2], in_=kn_c[:, 1:2])
    nc.gpsimd.dma_start(out=va_sb[:, 2:4], in_=va_c[:, 2:4])
    nc.gpsimd.dma_start(out=kn_sb[:, 2:4], in_=kn_c[:, 2:4])
    nc.gpsimd.dma_start(out=va_sb[:, 4:8], in_=va_c[:, 4:8])
    nc.gpsimd.dma_start(out=kn_sb[:, 4:8], in_=kn_c[:, 4:8])

    nc.vector.memset(scratch[:], 0.0)

    with (
        tc.tile_pool(name="psQK", bufs=2, space="PSUM") as psQK,
        tc.tile_pool(name="psH", bufs=2, space="PSUM") as psH,
        tc.tile_pool(name="psW", bufs=1, space="PSUM") as psW,
        tc.tile_pool(name="psWarm", bufs=1, space="PSUM") as psWarm,
        tc.tile_pool(name="work", bufs=2) as work,
    ):
        # all PSUM tiles are exact bank multiples so tiles never share a
        # bank (a matmul start=True clears the whole bank's has_written)
        psum_W = psW.tile([128, 512], F32)
        wview = psum_W[0:64, 0:HPC * DA].rearrange("p (h d) -> p h d", h=HPC)

        # HAM warm-up: the PE clock sits at 1.2 GHz until ~3.4us of
        # sustained matmul activity.  Spend the DMA-bound prologue on
        # dummy matmuls so real work starts (and stays) at 2.4 GHz.
        warm = psWarm.tile([128, 512], F32)
        for _ in range(10):
            nc.tensor.matmul(warm[:], lhsT=scratch[:, 0:128], rhs=scratch[:],
                             start=True, stop=True, skip_group_check=True)

        def emit_pqk(r):
            pq = psQK.tile([128, 1024], F32, name="pqk")
            for h in range(HPC):
                nc.tensor.matmul(pq[:, h * 128:(h + 1) * 128],
                                 lhsT=qs_sb[:, r, 2 * h + 1, :],
                                 rhs=qs_sb[:, r, 2 * h, :],
                                 start=True, stop=True, skip_group_check=True)
            return pq

        def emit_cp(pq):
            t = work.tile([128, HPC, 128], BF16, name="cp")
            nc.vector.tensor_tensor(
                out=t[:],
                in0=pq[:, 0:HPC * 128].rearrange("p (h s) -> p h s", h=HPC),
                in1=mk_sb[:].unsqueeze(1).broadcast_to([128, HPC, 128]),
                op=OP.mult)
            return t

        cp_cur = emit_cp(emit_pqk(0))
        wsb_prev = None

        for r in range(NCH):
            if r < 6:
                # bridge PE idle gaps so the HAM clock gate never sees an
                # idle window and re-throttles to 1.2GHz
                for _ in range(2):
                    nc.tensor.matmul(warm[:], lhsT=scratch[:, 0:128],
                                     rhs=scratch[:], start=True, stop=True,
                                     skip_group_check=True)
            if r + 1 < NCH:
                pq_n = emit_pqk(r + 1)       # tensor works ahead one chunk
                cp_nxt = emit_cp(pq_n)
            ph = psH.tile([128, 512], F32, name="ph")
            phv = ph[:, 0:HPC * DA].rearrange("p (h d) -> p h d", h=HPC)
            va_r = va_sb[:, r].rearrange("p (h d) -> p h d", h=HPC)
            if r > 0:
                # inter-chunk: ph = q @ W_{<r}; h==0 claims the bank
                for h in range(HPC):
                    nc.tensor.matmul(phv[:, h, :],
                                     lhsT=qs_sb[:, r, 2 * h, :],
                                     rhs=wsb_prev[:, h, :],
                                     start=(h == 0), stop=False,
                                     skip_group_check=True)
            for h in range(HPC):
                nc.tensor.matmul(phv[:, h, :], lhsT=cp_cur[:, h, :],
                                 rhs=va_r[:, h, :],
                                 start=(r == 0 and h == 0), stop=True,
                                 skip_group_check=True)
            if r + 1 < NCH:
                # (the last chunk's state update is dead work: W unread)
                for h in range(HPC):
                    nc.tensor.matmul(wview[:, h, :],
                                     lhsT=kn_sb[:, r, h * DH:(h + 1) * DH],
                                     rhs=va_r[:, h, :],
                                     start=(r == 0 and h == 0),
                                     stop=(r == NCH - 2), skip_group_check=True)
                wsb = work.tile([64, HPC, DA], BF16, name="wsb")
                nc.scalar.activation(out=wsb[:], in_=wview[:], func=AF.Copy)
            phsb = work.tile([128, HPC * DA], BF16, name="phsb")
            if r + 1 < NCH:
                nc.vector.tensor_copy(out=phsb[:], in_=ph[:, 0:HPC * DA])
                nc.sync.dma_start(out=out_c[:, r], in_=phsb[:])
            else:
                # split the final drain + out DMA for a shorter tail
                nc.vector.tensor_copy(out=phsb[:, 0:3 * DA],
                                      in_=ph[:, 0:3 * DA])
                nc.sync.dma_start(out=out_c[:, r, 0:3 * DA],
                                  in_=phsb[:, 0:3 * DA])
                nc.vector.tensor_copy(out=phsb[:, 3 * DA:HPC * DA],
                                      in_=ph[:, 3 * DA:HPC * DA])
                nc.sync.dma_start(out=out_c[:, r, 3 * DA:HPC * DA],
                                  in_=phsb[:, 3 * DA:HPC * DA])
            if r + 1 < NCH:
                cp_cur, wsb_prev = cp_nxt, wsb


_CACHED_NC = None


def _get_nc():
    global _CACHED_NC
    if _CACHED_NC is None:
        _CACHED_NC = build_nc()
    return _CACHED_NC


def _host_gates(q, k, v, igate_w, igate_b, fgate_w, fgate_b):
    """O(S) gate/scan work on host: returns em (bf16-ready), e2/emp, eps/emp."""
    x = np.concatenate([q, k, v], axis=2).reshape(-1, 3 * DIM)   # f32 gemm
    ig = (x @ igate_w.T).reshape(B, S, NH).astype(np.float64) + igate_b
    fg = (x @ fgate_w.T).reshape(B, S, NH).astype(np.float64) + fgate_b
    ls = -np.logaddexp(0.0, -fg)                 # log sigmoid
    L = np.cumsum(ls, axis=1)
    m = ig - L
    Mx = np.maximum.accumulate(m, axis=1)
    cH = Mx[:, -1:, :]
    em = np.exp(m - cH) * 0.125                  # <= 0.125, no overflow
    e2e = np.exp(-L - cH)                        # e2/emp (exponent bounded)
    epse = 1e-6 * np.exp(Mx - cH)                # eps/emp <= 1e-6
    return em, e2e, epse


_MASK_HOST = np.ascontiguousarray(
    np.tril(np.ones((128, 128), np.float32)).T).astype(ml_dtypes.bfloat16)


def _prep_core(q, k, v, em, b, g):
    hs = slice(HPC * g, HPC * g + HPC)
    qh = q[b].reshape(S, NH, DH)[:, hs]          # [S, 6, 64]
    kh = k[b].reshape(S, NH, DH)[:, hs]
    vh = v[b].reshape(S, NH, DH)[:, hs]
    qk2 = np.stack([qh, kh], axis=2)             # [S, 6, 2, 64]
    qs_host = np.ascontiguousarray(
        qk2.reshape(NCH, 128, HPC, 2, DH).transpose(4, 0, 2, 3, 1)
    ).reshape(64, -1).astype(ml_dtypes.bfloat16)
    kn_host = np.ascontiguousarray(
        kh.reshape(NCH, 128, HPC * DH).transpose(1, 0, 2)
    ).reshape(128, -1).astype(ml_dtypes.bfloat16)
    va = np.ones((NCH, 128, HPC, DA), np.float32)
    va[..., :DH] = vh.reshape(NCH, 128, HPC, DH)
    va *= em[b][:, hs].reshape(NCH, 128, HPC, 1)   # fold 0.125*exp(m-cH)
    va_host = np.ascontiguousarray(
        va.transpose(1, 0, 2, 3)).reshape(128, -1).astype(ml_dtypes.bfloat16)
    return {"qs": qs_host, "kn": kn_host, "va": va_host, "mk": _MASK_HOST}


_LAST_RESULT = {}


def kernel(q, k, v, igate_w, igate_b, fgate_w, fgate_b, norm_w, norm_b,
           **run_kwargs):
    nc = _get_nc()
    em, e2e, epse = _host_gates(q, k, v, igate_w, igate_b, fgate_w, fgate_b)
    in_maps = [_prep_core(q, k, v, em, core // 2, core % 2)
               for core in range(8)]

    res = run_bass_kernel_spmd(nc, in_maps, core_ids=list(range(8)),
                               **run_kwargs)
    _LAST_RESULT["res"] = res

    out = np.zeros((B, S, NH, DH), np.float32)
    for core in range(8):
        b, g = core // 2, core % 2
        hs = slice(HPC * g, HPC * g + HPC)
        o = np.asarray(res.results[core]["out"], dtype=np.float64)
        o = o.reshape(128, NCH, HPC, DA).transpose(1, 0, 2, 3).reshape(
            S, HPC, DA)
        ph_v, ph_rs = o[:, :, :DH], o[:, :, DH]
        sc = 1.0 / (np.maximum(np.abs(ph_rs), e2e[b][:, hs]) + epse[b][:, hs])
        h = ph_v * sc[..., None]
        mean = h.mean(-1, keepdims=True)
        var = ((h - mean) ** 2).mean(-1, keepdims=True)
        out[b, :, hs] = (h - mean) / np.sqrt(var + 1e-5)

    out = out.reshape(B, S, DIM)
    if np.any(norm_w) or np.any(norm_b):
        out = out * (1.0 + norm_w)[None, None, :] + norm_b[None, None, :]
    return out
